# revision 36
# baseline (speedup 1.0000x reference)
"""Causal self-attention (S=2048, D=1024, H=16) on 8 Trainium2 NeuronCores.

Sharding: tensor-parallel over heads. Core c owns heads 2c, 2c+1:
  - computes qT/kT/vT for its 128 qkv-columns from the full hidden_states
    (contraction layouts; vT is PE-transposed back to natural [s, j]),
  - runs causal attention for its 2 heads (attT = K.Q^T blocks, exp via
    ScalarE, denominators via a ones-column in the PV matmul),
  - projects each head against its W_proj row-slice and fuses the softmax
    normalization into the projection epilogue (per-partition 1/den scales
    computed with a custom-DVE fast reciprocal and broadcast with K=1
    matmuls from partitions 0/64),
  - outputs a partial [S, D] product; the host sums the 8 partials and
    adds b_proj.

The bulk matmuls run in bf16 (1 cycle/row); the unnormalized attention
outputs and denominators stay in fp32. Phase 2 is software-pipelined
(logits of group g overlap the PV matmuls of group g-1); the projection
(phase 3) is spread through phase 2 as soon as each query chunk's
normalizer is ready, so the PE stays dense (HAM stays at full clock) and
the output DMA overlaps compute instead of trailing it.
"""

import math
from contextlib import ExitStack

import numpy as np

import concourse.bacc as bacc
import concourse.hw_specs as _hw_specs
import concourse.mybir as mybir
import concourse.tile as tile
from concourse.bass_utils import run_bass_kernel_spmd

# The kernel's only ScalarE activations are Exp and Ln. Left to itself, bass
# assigns Exp to the `exp_and_others` table set and Ln to `natural_log`, so
# every Exp->Ln alternation reloads the activation tables (~1.3us, 9x per
# kernel, serializing ScalarE mid-attention). Restrict the selectable sets to
# `natural_log_exp_and_others` (which contains both) so exactly one table
# load is emitted. Set ids keep their act_info.json positions.
_orig_gat = _hw_specs.get_activation_tables


def _pinned_gat(arch):
    return {
        name: (fns if name == "natural_log_exp_and_others" else set())
        for name, fns in _orig_gat(arch).items()
    }


bacc.get_activation_tables = _pinned_gat

S, D, H = 2048, 1024, 16
HS = D // H  # 64 head size
P = 128
NCORES = 8
HPC = H // NCORES  # 2 heads per core
CD = HPC * HS  # 128 per-core head dims
KO = D // P  # 8 contraction tiles for the projections
NQC = S // 512  # 4 query chunks
NSC = S // P  # 16 sequence chunks of 128
SCALE = 1.0 / math.sqrt(S)

F32 = mybir.dt.float32
F32R = mybir.dt.float32r
BF16 = mybir.dt.bfloat16

try:
    import ml_dtypes

    NP_BF16 = ml_dtypes.bfloat16
except ImportError:  # pragma: no cover
    NP_BF16 = None


def _build():
    nc = bacc.Bacc(
        "TRN2", target_bir_lowering=False, debug=False, num_devices=NCORES
    )

    hsT = nc.dram_tensor("hsT", [D, S], BF16, kind="ExternalInput")
    w_qkv = nc.dram_tensor("w_qkv", [D, 3 * P], BF16, kind="ExternalInput")
    b_qkv = nc.dram_tensor("b_qkv", [P, 3], F32, kind="ExternalInput")
    w_p = nc.dram_tensor("w_p", [CD, D], BF16, kind="ExternalInput")
    msk = nc.dram_tensor("msk", [P, 896], BF16, kind="ExternalInput")
    iden_b = nc.dram_tensor("iden_b", [P, P], BF16, kind="ExternalInput")
    esel = nc.dram_tensor("esel", [65, P], BF16, kind="ExternalInput")
    out = nc.dram_tensor("out", [S, D], BF16, kind="ExternalOutput")

    with (
        tile.TileContext(nc) as tc,
        ExitStack() as ctx,
        nc.allow_low_precision(reason="bf16 matmul pipeline"),
    ):
        const = ctx.enter_context(tc.tile_pool(name="const", bufs=1))
        work = ctx.enter_context(tc.tile_pool(name="work", bufs=2))
        pp = ctx.enter_context(tc.tile_pool(name="pp", bufs=1, space="PSUM"))

        # PSUM: 2x 2-bank slots for attention logits (the two heads of a
        # group alternate slots, so each head's next logits launch while the
        # other head's exp drains), 2x 1-bank slots for everything else
        # (qkv passes, v transposes, projection halves, normalizer
        # broadcast), 2 banks for the per-chunk PV accumulators = all 8
        def att_ps(name):
            return pp.tile([P, 2, 512], F32, tag="attA", bufs=2, name=name)

        def b_ps(name):
            return pp.tile([P, 512], F32, tag="B", bufs=2, name=name)

        # ---- loads: identity first (pre-warm), then per-o weight+hsT chunks
        identb = const.tile([P, P], BF16, tag="identb", name="identb")
        nc.sync.dma_start(out=identb, in_=iden_b.ap())
        esel_sb = const.tile([65, P], BF16, tag="esel", name="esel_sb")
        nc.gpsimd.dma_start(out=esel_sb, in_=esel.ap())
        bqkv_sb = const.tile([P, 3], F32, tag="bqkv", name="bqkv_sb")
        nc.sync.dma_start(out=bqkv_sb, in_=b_qkv.ap())

        # input loads ride three DMA queues: sync + gpsimd + scalar (the
        # scalar engine is a HWDGE and sits idle until the first exp). The
        # phase-1 critical inputs (hsT columns 0:512 + the qkv weights) get
        # dedicated queues so nothing delays the first attention chunk.
        hsT_sb = const.tile([P, KO, S], BF16, tag="hsT", name="hsT_sb")
        wqkv_sb = const.tile([P, KO, 3 * P], BF16, tag="wqkv", name="wqkv_sb")
        for o in range(KO):
            nc.gpsimd.dma_start(
                out=wqkv_sb[:, o, :], in_=w_qkv.ap()[o * P : (o + 1) * P, :]
            )
            nc.sync.dma_start(
                out=hsT_sb[:, o, 0:512], in_=hsT.ap()[o * P : (o + 1) * P, 0:512]
            )
        msk_sb = const.tile([P, 896], BF16, tag="msk", name="msk_sb")
        nc.scalar.dma_start(out=msk_sb, in_=msk.ap())
        wp_sb = const.tile([P, D], BF16, tag="wp", name="wp_sb")
        nc.scalar.dma_start(out=wp_sb, in_=w_p.ap())
        v_sb = []
        for h in range(HPC):
            vt = const.tile([P, NSC, HS + 1], BF16, tag=f"v{h}", name=f"v{h}_sb")
            nc.vector.memset(vt[:, :, HS], 1.0)  # ones column -> denominators
            v_sb.append(vt)
        # remaining hsT columns: wide (1.5KB-line) DMAs round-robin on 3 queues
        dma_rr = [nc.sync, nc.gpsimd, nc.scalar]
        for o in range(KO):
            dma_rr[(2 * o) % 3].dma_start(
                out=hsT_sb[:, o, 512:1280],
                in_=hsT.ap()[o * P : (o + 1) * P, 512:1280],
            )
            dma_rr[(2 * o + 1) % 3].dma_start(
                out=hsT_sb[:, o, 1280:2048],
                in_=hsT.ap()[o * P : (o + 1) * P, 1280:2048],
            )

        # pre-load the exp/ln activation table during the input DMA window
        # instead of at first use mid-attention (emitted after the scalar
        # queue's input DMA triggers so the table load doesn't delay them)
        scratch8 = const.tile([1, 8], F32, tag="scratch8", name="scratch8")
        nc.vector.memset(scratch8, 1.0)
        nc.scalar.activation(
            out=scratch8, in_=scratch8, func=mybir.ActivationFunctionType.Exp
        )

        qkT_sb = const.tile([P, 2, S], BF16, tag="qkT", name="qkT_sb")
        vT_sb = const.tile([P, S], BF16, tag="vT", name="vT_sb")
        u2_sb = [
            const.tile([P, 512], BF16, tag=f"u2_{qc}", name=f"u2_{qc}")
            for qc in range(NQC)
        ]
        u2n_sb = [
            const.tile([P, 512], BF16, tag=f"u2n_{qc}", name=f"u2n_{qc}")
            for qc in range(NQC)
        ]
        # softmax denominators for both heads: head h lives on partition 64h;
        # rows 1-63 are never written, so pre-fill with 1.0 to keep NaNs out
        # of the reciprocal -> selector-matmul path
        den_sb = []
        for qc in range(NQC):
            dt_ = const.tile([65, 512], F32, tag=f"den_{qc}", name=f"den_{qc}")
            nc.vector.memset(dt_, 1.0)
            den_sb.append(dt_)

        # ---- pre-warm the PE clock while the DMAs stream ---------------------
        # each burst consumes a freshly-arrived hsT chunk so the bursts are
        # spread across the load instead of back-to-back at t=0
        ps_w = b_ps("ps_w")
        for o in range(KO):
            nc.tensor.matmul(
                ps_w,
                lhsT=identb,
                rhs=hsT_sb[:, o, 0:512],
                start=True,
                stop=True,
            )

        # ---- phase 1: qT, kT, vT ([j, s] layout) + v transposes -------------
        # split into 7 pieces per 512-chunk (3 qkv passes + 4 v transposes)
        # that get sprinkled between attention groups: the PE FIFO then
        # alternates phase-1 and attention work, so ScalarE's exp stream
        # never starves behind a phase-1 lump at chunk boundaries
        def emit_p1_m(n, m):
            ps_qkv = b_ps("ps_qkv")
            for o in range(KO):
                nc.tensor.matmul(
                    ps_qkv,
                    lhsT=wqkv_sb[:, o, m * P : (m + 1) * P],
                    rhs=hsT_sb[:, o, n * 512 : (n + 1) * 512],
                    start=(o == 0),
                    stop=(o == KO - 1),
                )
            dst = (
                qkT_sb[:, m, n * 512 : (n + 1) * 512]
                if m < 2
                else vT_sb[:, n * 512 : (n + 1) * 512]
            )
            nc.vector.tensor_scalar_add(
                out=dst, in0=ps_qkv, scalar1=bqkv_sb[:, m : m + 1]
            )

        def emit_p1_t(n, i):
            sc = 4 * n + i
            ps_t = pp.tile([P, P], BF16, tag="B", bufs=2, name="ps_t")
            nc.tensor.transpose(ps_t, vT_sb[:, sc * P : (sc + 1) * P], identb)
            for h in range(HPC):
                nc.vector.tensor_copy(
                    out=v_sb[h][:, sc, 0:HS], in_=ps_t[:, h * HS : (h + 1) * HS]
                )

        def p1_pieces(n):
            for m in range(3):
                yield lambda m=m: emit_p1_m(n, m)
            for i in range(4):
                yield lambda i=i: emit_p1_t(n, i)

        for piece in p1_pieces(0):
            piece()

        def emit_norm(qc):
            # 1/den = exp(-ln(den)) on ScalarE, both heads in one [65,512]
            # pass (the pinned table set holds Exp AND Ln: no table reloads);
            # one K=65 selector matmul broadcasts head h's reciprocal row to
            # its 64 partitions; one multiply normalizes both heads
            lnw = work.tile([65, 512], F32, tag="lnw", bufs=2, name="lnw")
            nc.scalar.activation(
                out=lnw, in_=den_sb[qc], func=mybir.ActivationFunctionType.Ln
            )
            rrec = work.tile([65, 512], BF16, tag="rrec", bufs=2, name="rrec")
            nc.scalar.activation(
                out=rrec,
                in_=lnw,
                func=mybir.ActivationFunctionType.Exp,
                scale=-1.0,
            )
            rb_ps = b_ps("ps_rb")
            nc.tensor.matmul(
                rb_ps, lhsT=esel_sb, rhs=rrec, start=True, stop=True
            )
            nc.vector.tensor_mul(out=u2n_sb[qc], in0=u2_sb[qc], in1=rb_ps)

        # ---- phase 3: projection over both heads (K=128), two 1-bank halves
        # per chunk so the B pool recycles quickly
        def emit_p3(sc, tail=False):
            qc = sc // 4
            f = sc % 4
            out_t = work.tile([P, 2, 512], BF16, tag="out", bufs=4, name="out_t")
            for dc in range(2):
                slot = b_ps("ps_p3")
                nc.tensor.matmul(
                    slot,
                    lhsT=u2n_sb[qc][:, f * P : (f + 1) * P],
                    rhs=wp_sb[:, dc * 512 : (dc + 1) * 512],
                    start=True,
                    stop=True,
                )
                # in the tail the scalar engine is idle (no more exps): give
                # it half the PSUM evacuations
                if tail and dc == 0:
                    nc.scalar.copy(out=out_t[:, dc, :], in_=slot)
                else:
                    nc.vector.tensor_copy(out=out_t[:, dc, :], in_=slot)
            eng = (nc.sync, nc.gpsimd, nc.scalar)[sc % 3 if tail else sc % 2]
            eng.dma_start(
                out=out.ap()[sc * P : (sc + 1) * P, :],
                in_=out_t.rearrange("p a b -> p (a b)"),
            )

        # ---- phase 2: causal attention, software-pipelined ------------------
        for qc in range(NQC):
            ps_o = [
                pp.tile([P, 512], F32, tag="O", bufs=2, name=f"ps_o{h}")
                for h in range(HPC)
            ]
            nkb = 4 * (qc + 1)  # 128-wide key blocks in the causal span
            ngrp = nkb // 2
            # previous chunk's normalizer first: it enters the ScalarE FIFO
            # ahead of this chunk's exps, so its projection chunks (spread
            # below) overlap this chunk's attention instead of piling up at
            # the end of the kernel
            if qc >= 1:
                emit_norm(qc - 1)
            pieces = list(p1_pieces(qc + 1)) if qc + 1 < NQC else []
            npc = 0

            def emit_pv(pend, nkb=nkb, ps_o=ps_o):
                pes, kbs, f0 = pend
                for h in range(HPC):
                    for j, kb in enumerate(kbs):
                        nc.tensor.matmul(
                            ps_o[h][0 : HS + 1, f0:512],
                            lhsT=v_sb[h][:, kb, :],
                            rhs=pes[h][:, j, f0:512],
                            start=(kb == 0),
                            stop=(kb == nkb - 1),
                        )

            pending = None  # exp'd logits awaiting their PV matmuls
            for g in range(ngrp):
                kbs = [2 * g, 2 * g + 1]
                # last group covers only the causal upper half of the q range
                f0 = 256 if g == ngrp - 1 else 0
                # logits for both heads; explicit row-group tile positions
                # let the two K=64 matmuls run concurrently in disjoint
                # halves of the PE array
                ps_att = [att_ps(f"ps_att{h}") for h in range(HPC)]
                for j, kb in enumerate(kbs):
                    for h in range(HPC):
                        nc.tensor.matmul(
                            ps_att[h][:, j, f0:512],
                            lhsT=qkT_sb[h * HS : (h + 1) * HS, 1, kb * P : (kb + 1) * P],
                            rhs=qkT_sb[h * HS : (h + 1) * HS, 0, qc * 512 + f0 : (qc + 1) * 512],
                            start=True,
                            stop=True,
                            tile_position=(HS * h, 0),
                        )
                if pending is not None:
                    emit_pv(pending)
                pes = []
                for h in range(HPC):
                    p_exp = work.tile(
                        [P, 2, 512], BF16, tag=f"pe{h}", bufs=4, name="p_exp"
                    )
                    nc.scalar.activation(
                        out=p_exp[:, :, f0:512],
                        in_=ps_att[h][:, :, f0:512],
                        func=mybir.ActivationFunctionType.Exp,
                        scale=SCALE,
                    )
                    for j, kb in enumerate(kbs):
                        jj = kb - 4 * qc
                        if jj >= 0:  # diagonal block: causal 0/1 mask
                            off = 384 - 128 * jj
                            nc.vector.tensor_mul(
                                out=p_exp[:, j, f0:512],
                                in0=p_exp[:, j, f0:512],
                                in1=msk_sb[:, off + f0 : off + 512],
                            )
                    pes.append(p_exp)
                pending = (pes, kbs, f0)
                if qc >= 1 and g < 4:
                    emit_p3(4 * (qc - 1) + g)
                while npc < ((g + 1) * len(pieces)) // ngrp:
                    pieces[npc]()
                    npc += 1
            emit_pv(pending)

            # stash denominator rows first (they gate the next normalizer on
            # ScalarE), then the unnormalized head outputs; frees PSUM
            for h in range(HPC):
                nc.vector.tensor_copy(
                    out=den_sb[qc][64 * h : 64 * h + 1, :],
                    in_=ps_o[h][HS : HS + 1, :],
                )
            for h in range(HPC):
                nc.vector.tensor_copy(
                    out=u2_sb[qc][h * HS : (h + 1) * HS, :], in_=ps_o[h][0:HS, :]
                )
        # keep the PE busy through the final normalizer chain so the tail
        # projections run at full clock (HAM stays at 8/8)
        ps_warm = b_ps("ps_warm")
        for rep in range(6):
            nc.tensor.matmul(
                ps_warm,
                lhsT=identb,
                rhs=hsT_sb[:, rep, 0:512],
                start=True,
                stop=True,
            )
        emit_norm(3)
        for sc in range(12, NSC):
            emit_p3(sc, tail=True)

    nc.compile()
    return nc


_NC = None


def _get_nc():
    global _NC
    if _NC is None:
        _NC = _build()
    return _NC


def prepare_inputs(hidden_states, W_attn, b_attn, W_proj, b_proj):
    hs = np.asarray(hidden_states, dtype=np.float32)
    Wa = np.asarray(W_attn, dtype=np.float32)
    ba = np.asarray(b_attn, dtype=np.float32)
    Wp = np.asarray(W_proj, dtype=np.float32)

    hsT = np.ascontiguousarray(hs.T).astype(NP_BF16)
    pcol = np.arange(P)[:, None]
    ccol = np.arange(896)[None, :]
    msk = (pcol <= ccol - 384).astype(NP_BF16)
    esel = np.zeros((65, P), dtype=np.float32)
    esel[0, 0:HS] = 1.0
    esel[64, HS:P] = 1.0
    esel = esel.astype(NP_BF16)

    in_maps = []
    for c in range(NCORES):
        q0 = c * CD
        wq = Wa[:, q0 : q0 + CD]
        wk = Wa[:, D + q0 : D + q0 + CD]
        wv = Wa[:, 2 * D + q0 : 2 * D + q0 + CD]
        bq = ba[q0 : q0 + CD]
        bk = ba[D + q0 : D + q0 + CD]
        bv = ba[2 * D + q0 : 2 * D + q0 + CD]
        in_maps.append(
            {
                "hsT": hsT,
                "w_qkv": np.ascontiguousarray(
                    np.concatenate([wq, wk, wv], axis=1)
                ).astype(NP_BF16),
                "b_qkv": np.ascontiguousarray(np.stack([bq, bk, bv], axis=1)).astype(
                    np.float32
                ),
                "w_p": np.ascontiguousarray(Wp[q0 : q0 + CD, :]).astype(NP_BF16),
                "msk": msk,
                "iden_b": np.eye(P).astype(NP_BF16),
                "esel": esel,
            }
        )
    return in_maps


def run(inputs, trace=False):
    """Build+run the sharded kernel. Returns (full_output, BassKernelResults)."""
    in_maps = prepare_inputs(**inputs)
    nc = _get_nc()
    res = run_bass_kernel_spmd(
        nc, in_maps, core_ids=list(range(NCORES)), trace=trace
    )
    acc = np.zeros((S, D), dtype=np.float32)
    for c in range(NCORES):
        acc += np.asarray(res.results[c]["out"], dtype=np.float32)
    acc += np.asarray(inputs["b_proj"], dtype=np.float32)
    return acc, res


def kernel(**inputs):
    out, _ = run(inputs, trace=False)
    return out


# revision 42
# speedup vs baseline: 1.1620x; 1.1620x over previous
"""Causal self-attention (S=2048, D=1024, H=16) on 8 Trainium2 NeuronCores.

Sharding: tensor-parallel over heads. Core c owns heads 2c, 2c+1:
  - computes qT/kT/vT for its 128 qkv-columns from the full hidden_states
    (contraction layouts; vT is PE-transposed back to natural [s, j]),
  - runs causal attention for its 2 heads (attT = K.Q^T blocks, exp via
    ScalarE, denominators via a ones-column in the PV matmul),
  - projects each head against its W_proj row-slice and fuses the softmax
    normalization into the projection epilogue (per-partition 1/den scales
    computed with a custom-DVE fast reciprocal and broadcast with K=1
    matmuls from partitions 0/64),
  - outputs a partial [S, D] product; the host sums the 8 partials and
    adds b_proj.

The bulk matmuls run in bf16 (1 cycle/row); the unnormalized attention
outputs and denominators stay in fp32. Phase 2 is software-pipelined
(logits of group g overlap the PV matmuls of group g-1); the projection
(phase 3) is spread through phase 2 as soon as each query chunk's
normalizer is ready, so the PE stays dense (HAM stays at full clock) and
the output DMA overlaps compute instead of trailing it.
"""

import math
from contextlib import ExitStack

import numpy as np

import concourse.bacc as bacc
import concourse.hw_specs as _hw_specs
import concourse.mybir as mybir
import concourse.tile as tile
from concourse.bass_utils import run_bass_kernel_spmd

# The kernel's only ScalarE activations are Exp and Ln. Left to itself, bass
# assigns Exp to the `exp_and_others` table set and Ln to `natural_log`, so
# every Exp->Ln alternation reloads the activation tables (~1.3us, 9x per
# kernel, serializing ScalarE mid-attention). Restrict the selectable sets to
# `natural_log_exp_and_others` (which contains both) so exactly one table
# load is emitted. Set ids keep their act_info.json positions.
_orig_gat = _hw_specs.get_activation_tables


def _pinned_gat(arch):
    return {
        name: (fns if name == "natural_log_exp_and_others" else set())
        for name, fns in _orig_gat(arch).items()
    }


bacc.get_activation_tables = _pinned_gat

S, D, H = 2048, 1024, 16
HS = D // H  # 64 head size
P = 128
NCORES = 8
HPC = H // NCORES  # 2 heads per core
CD = HPC * HS  # 128 per-core head dims
KO = D // P  # 8 contraction tiles for the projections
NQC = S // 512  # 4 query chunks
NSC = S // P  # 16 sequence chunks of 128
SCALE = 1.0 / math.sqrt(S)

F32 = mybir.dt.float32
F32R = mybir.dt.float32r
BF16 = mybir.dt.bfloat16

try:
    import ml_dtypes

    NP_BF16 = ml_dtypes.bfloat16
except ImportError:  # pragma: no cover
    NP_BF16 = None


def _build():
    nc = bacc.Bacc(
        "TRN2", target_bir_lowering=False, debug=False, num_devices=NCORES
    )

    hsT = nc.dram_tensor("hsT", [D, S], BF16, kind="ExternalInput")
    w_qkv = nc.dram_tensor("w_qkv", [D, 3 * P], BF16, kind="ExternalInput")
    b_qkv = nc.dram_tensor("b_qkv", [P, 3], F32, kind="ExternalInput")
    w_p = nc.dram_tensor("w_p", [CD, D], BF16, kind="ExternalInput")
    msk = nc.dram_tensor("msk", [P, 896], BF16, kind="ExternalInput")
    iden_b = nc.dram_tensor("iden_b", [P, P], BF16, kind="ExternalInput")
    esel = nc.dram_tensor("esel", [65, P], BF16, kind="ExternalInput")
    out = nc.dram_tensor("out", [S, D], F32, kind="ExternalOutput")

    with (
        tile.TileContext(nc) as tc,
        ExitStack() as ctx,
        nc.allow_low_precision(reason="bf16 matmul pipeline"),
    ):
        const = ctx.enter_context(tc.tile_pool(name="const", bufs=1))
        work = ctx.enter_context(tc.tile_pool(name="work", bufs=2))
        pp = ctx.enter_context(tc.tile_pool(name="pp", bufs=1, space="PSUM"))

        # PSUM: 2x 2-bank slots for attention logits (the two heads of a
        # group alternate slots, so each head's next logits launch while the
        # other head's exp drains), 2x 1-bank slots for everything else
        # (qkv passes, v transposes, projection halves, normalizer
        # broadcast), 2 banks for the per-chunk PV accumulators = all 8
        def att_ps(name):
            return pp.tile([P, 2, 512], F32, tag="attA", bufs=2, name=name)

        def b_ps(name):
            return pp.tile([P, 512], F32, tag="B", bufs=2, name=name)

        # ---- loads: identity first (pre-warm), then per-o weight+hsT chunks
        identb = const.tile([P, P], BF16, tag="identb", name="identb")
        nc.sync.dma_start(out=identb, in_=iden_b.ap())
        esel_sb = const.tile([65, P], BF16, tag="esel", name="esel_sb")
        nc.gpsimd.dma_start(out=esel_sb, in_=esel.ap())
        bqkv_sb = const.tile([P, 3], F32, tag="bqkv", name="bqkv_sb")
        nc.sync.dma_start(out=bqkv_sb, in_=b_qkv.ap())

        # input loads ride three DMA queues: sync + gpsimd + scalar (the
        # scalar engine is a HWDGE and sits idle until the first exp). The
        # phase-1 critical inputs (hsT columns 0:512 + the qkv weights) get
        # dedicated queues so nothing delays the first attention chunk.
        hsT_sb = const.tile([P, KO, S], BF16, tag="hsT", name="hsT_sb")
        wqkv_sb = const.tile([P, KO, 3 * P], BF16, tag="wqkv", name="wqkv_sb")
        for o in range(KO):
            nc.gpsimd.dma_start(
                out=wqkv_sb[:, o, :], in_=w_qkv.ap()[o * P : (o + 1) * P, :]
            )
            eng = nc.sync if o % 2 == 0 else nc.scalar
            eng.dma_start(
                out=hsT_sb[:, o, 0:512], in_=hsT.ap()[o * P : (o + 1) * P, 0:512]
            )
        msk_sb = const.tile([P, 896], BF16, tag="msk", name="msk_sb")
        nc.sync.dma_start(out=msk_sb, in_=msk.ap())
        wp_sb = const.tile([P, D], BF16, tag="wp", name="wp_sb")
        nc.gpsimd.dma_start(out=wp_sb, in_=w_p.ap())
        v_sb = []
        for h in range(HPC):
            vt = const.tile([P, NSC, HS + 1], BF16, tag=f"v{h}", name=f"v{h}_sb")
            nc.vector.memset(vt[:, :, HS], 1.0)  # ones column -> denominators
            v_sb.append(vt)
        # remaining hsT columns: wide (1.5KB-line) DMAs round-robin on 3 queues
        dma_rr = [nc.sync, nc.gpsimd, nc.scalar]
        for o in range(KO):
            dma_rr[(2 * o) % 3].dma_start(
                out=hsT_sb[:, o, 512:1280],
                in_=hsT.ap()[o * P : (o + 1) * P, 512:1280],
            )
            dma_rr[(2 * o + 1) % 3].dma_start(
                out=hsT_sb[:, o, 1280:2048],
                in_=hsT.ap()[o * P : (o + 1) * P, 1280:2048],
            )

        # pre-load the exp/ln activation table during the input DMA window
        # instead of at first use mid-attention (emitted after the scalar
        # queue's input DMA triggers so the table load doesn't delay them)
        scratch8 = const.tile([1, 8], F32, tag="scratch8", name="scratch8")
        nc.vector.memset(scratch8, 1.0)
        nc.scalar.activation(
            out=scratch8, in_=scratch8, func=mybir.ActivationFunctionType.Exp
        )

        qkT_sb = const.tile([P, 2, S], BF16, tag="qkT", name="qkT_sb")
        vT_sb = const.tile([P, S], BF16, tag="vT", name="vT_sb")
        u2_sb = [
            const.tile([P, 512], BF16, tag=f"u2_{qc}", name=f"u2_{qc}")
            for qc in range(NQC)
        ]
        u2n_sb = [
            const.tile([P, 512], BF16, tag=f"u2n_{qc}", name=f"u2n_{qc}")
            for qc in range(NQC)
        ]
        # softmax denominators for both heads: head h lives on partition 64h;
        # rows 1-63 are never written, so pre-fill with 1.0 to keep NaNs out
        # of the reciprocal -> selector-matmul path
        den_sb = []
        for qc in range(NQC):
            dt_ = const.tile([65, 512], F32, tag=f"den_{qc}", name=f"den_{qc}")
            nc.vector.memset(dt_, 1.0)
            den_sb.append(dt_)

        # ---- pre-warm the PE clock while the DMAs stream ---------------------
        # each burst consumes a freshly-arrived hsT chunk so the bursts are
        # spread across the load instead of back-to-back at t=0
        ps_w = b_ps("ps_w")
        for o in range(KO):
            for rep in range(2):
                nc.tensor.matmul(
                    ps_w,
                    lhsT=identb,
                    rhs=hsT_sb[:, o, 0:512],
                    start=True,
                    stop=True,
                )

        # ---- phase 1: qT, kT, vT ([j, s] layout) + v transposes -------------
        # split into 7 pieces per 512-chunk (3 qkv passes + 4 v transposes)
        # that get sprinkled between attention groups: the PE FIFO then
        # alternates phase-1 and attention work, so ScalarE's exp stream
        # never starves behind a phase-1 lump at chunk boundaries
        def emit_p1_m(n, m):
            ps_qkv = b_ps("ps_qkv")
            for o in range(KO):
                nc.tensor.matmul(
                    ps_qkv,
                    lhsT=wqkv_sb[:, o, m * P : (m + 1) * P],
                    rhs=hsT_sb[:, o, n * 512 : (n + 1) * 512],
                    start=(o == 0),
                    stop=(o == KO - 1),
                )
            dst = (
                qkT_sb[:, m, n * 512 : (n + 1) * 512]
                if m < 2
                else vT_sb[:, n * 512 : (n + 1) * 512]
            )
            nc.vector.tensor_scalar_add(
                out=dst, in0=ps_qkv, scalar1=bqkv_sb[:, m : m + 1]
            )

        def emit_p1_t(n, i):
            sc = 4 * n + i
            ps_t = pp.tile([P, P], BF16, tag="B", bufs=2, name="ps_t")
            nc.tensor.transpose(ps_t, vT_sb[:, sc * P : (sc + 1) * P], identb)
            for h in range(HPC):
                nc.vector.tensor_copy(
                    out=v_sb[h][:, sc, 0:HS], in_=ps_t[:, h * HS : (h + 1) * HS]
                )

        def p1_pieces(n):
            for m in range(3):
                yield lambda m=m: emit_p1_m(n, m)
            for i in range(4):
                yield lambda i=i: emit_p1_t(n, i)

        for piece in p1_pieces(0):
            piece()

        def emit_norm(qc):
            # 1/den = exp(-ln(den)) on ScalarE, both heads in one [65,512]
            # pass (the pinned table set holds Exp AND Ln: no table reloads);
            # one K=65 selector matmul broadcasts head h's reciprocal row to
            # its 64 partitions; one multiply normalizes both heads
            lnw = work.tile([65, 512], F32, tag="lnw", bufs=2, name="lnw")
            nc.scalar.activation(
                out=lnw, in_=den_sb[qc], func=mybir.ActivationFunctionType.Ln
            )
            rrec = work.tile([65, 512], BF16, tag="rrec", bufs=2, name="rrec")
            nc.scalar.activation(
                out=rrec,
                in_=lnw,
                func=mybir.ActivationFunctionType.Exp,
                scale=-1.0,
            )
            rb_ps = b_ps("ps_rb")
            nc.tensor.matmul(
                rb_ps, lhsT=esel_sb, rhs=rrec, start=True, stop=True
            )
            nc.vector.tensor_mul(out=u2n_sb[qc], in0=u2_sb[qc], in1=rb_ps)

        # ---- phase 3: projection over both heads (K=128), two 1-bank halves
        # per chunk so the B pool recycles quickly
        def emit_p3(sc, tail=False):
            qc = sc // 4
            f = sc % 4
            out_t = work.tile([P, 2, 512], F32, tag="out", bufs=4, name="out_t")
            for dc in range(2):
                slot = b_ps("ps_p3")
                nc.tensor.matmul(
                    slot,
                    lhsT=u2n_sb[qc][:, f * P : (f + 1) * P],
                    rhs=wp_sb[:, dc * 512 : (dc + 1) * 512],
                    start=True,
                    stop=True,
                )
                # in the tail the scalar engine is idle (no more exps): give
                # it half the PSUM evacuations
                if tail and dc == 0:
                    nc.scalar.copy(out=out_t[:, dc, :], in_=slot)
                else:
                    nc.vector.tensor_copy(out=out_t[:, dc, :], in_=slot)
            eng = (nc.sync, nc.gpsimd, nc.scalar)[sc % 3 if tail else sc % 2]
            eng.dma_start(
                out=out.ap()[sc * P : (sc + 1) * P, :],
                in_=out_t.rearrange("p a b -> p (a b)"),
            )

        # ---- phase 2: causal attention, software-pipelined ------------------
        for qc in range(NQC):
            ps_o = [
                pp.tile([P, 512], F32, tag="O", bufs=2, name=f"ps_o{h}")
                for h in range(HPC)
            ]
            nkb = 4 * (qc + 1)  # 128-wide key blocks in the causal span
            ngrp = nkb // 2
            pieces = list(p1_pieces(qc + 1)) if qc + 1 < NQC else []
            npc = 0

            def emit_pv(pend, nkb=nkb, ps_o=ps_o):
                pes, kbs, f0 = pend
                for h in range(HPC):
                    for j, kb in enumerate(kbs):
                        nc.tensor.matmul(
                            ps_o[h][0 : HS + 1, f0:512],
                            lhsT=v_sb[h][:, kb, :],
                            rhs=pes[h][:, j, f0:512],
                            start=(kb == 0),
                            stop=(kb == nkb - 1),
                        )

            pending = None  # exp'd logits awaiting their PV matmuls
            for g in range(ngrp):
                kbs = [2 * g, 2 * g + 1]
                # last group covers only the causal upper half of the q range
                f0 = 256 if g == ngrp - 1 else 0
                # logits for both heads; explicit row-group tile positions
                # let the two K=64 matmuls run concurrently in disjoint
                # halves of the PE array
                ps_att = [att_ps(f"ps_att{h}") for h in range(HPC)]
                for j, kb in enumerate(kbs):
                    for h in range(HPC):
                        nc.tensor.matmul(
                            ps_att[h][:, j, f0:512],
                            lhsT=qkT_sb[h * HS : (h + 1) * HS, 1, kb * P : (kb + 1) * P],
                            rhs=qkT_sb[h * HS : (h + 1) * HS, 0, qc * 512 + f0 : (qc + 1) * 512],
                            start=True,
                            stop=True,
                            tile_position=(HS * h, 0),
                        )
                if pending is not None:
                    emit_pv(pending)
                pes = []
                for h in range(HPC):
                    p_exp = work.tile(
                        [P, 2, 512], BF16, tag=f"pe{h}", bufs=4, name="p_exp"
                    )
                    nc.scalar.activation(
                        out=p_exp[:, :, f0:512],
                        in_=ps_att[h][:, :, f0:512],
                        func=mybir.ActivationFunctionType.Exp,
                        scale=SCALE,
                    )
                    for j, kb in enumerate(kbs):
                        jj = kb - 4 * qc
                        if jj >= 0:  # diagonal block: causal 0/1 mask
                            off = 384 - 128 * jj
                            nc.vector.tensor_mul(
                                out=p_exp[:, j, f0:512],
                                in0=p_exp[:, j, f0:512],
                                in1=msk_sb[:, off + f0 : off + 512],
                            )
                    pes.append(p_exp)
                pending = (pes, kbs, f0)
                # the previous chunk's normalizer is emitted after group 0's
                # exps: they cover the ScalarE FIFO while its denominator
                # copies drain, so the Ln never stalls the exp stream. Its
                # four projection chunks follow, one per group.
                if qc >= 1:
                    if g == 0:
                        emit_norm(qc - 1)
                    elif g <= 4:
                        emit_p3(4 * (qc - 1) + g - 1)
                        if qc == 1 and g == ngrp - 1:
                            emit_p3(4 * (qc - 1) + g)
                while npc < ((g + 1) * len(pieces)) // ngrp:
                    pieces[npc]()
                    npc += 1
            emit_pv(pending)

            # stash denominator rows first (they gate the next normalizer on
            # ScalarE), then the unnormalized head outputs; frees PSUM
            for h in range(HPC):
                nc.vector.tensor_copy(
                    out=den_sb[qc][64 * h : 64 * h + 1, :],
                    in_=ps_o[h][HS : HS + 1, :],
                )
            for h in range(HPC):
                nc.vector.tensor_copy(
                    out=u2_sb[qc][h * HS : (h + 1) * HS, :], in_=ps_o[h][0:HS, :]
                )
        # keep the PE busy through the final normalizer chain so the tail
        # projections run at full clock (HAM stays at 8/8)
        ps_warm = b_ps("ps_warm")
        for rep in range(6):
            nc.tensor.matmul(
                ps_warm,
                lhsT=identb,
                rhs=hsT_sb[:, rep, 0:512],
                start=True,
                stop=True,
            )
        emit_norm(3)
        for sc in range(12, NSC):
            emit_p3(sc, tail=True)

    nc.compile()
    return nc


_NC = None


def _get_nc():
    global _NC
    if _NC is None:
        _NC = _build()
    return _NC


def prepare_inputs(hidden_states, W_attn, b_attn, W_proj, b_proj):
    hs = np.asarray(hidden_states, dtype=np.float32)
    Wa = np.asarray(W_attn, dtype=np.float32)
    ba = np.asarray(b_attn, dtype=np.float32)
    Wp = np.asarray(W_proj, dtype=np.float32)

    hsT = np.ascontiguousarray(hs.T).astype(NP_BF16)
    pcol = np.arange(P)[:, None]
    ccol = np.arange(896)[None, :]
    msk = (pcol <= ccol - 384).astype(NP_BF16)
    esel = np.zeros((65, P), dtype=np.float32)
    esel[0, 0:HS] = 1.0
    esel[64, HS:P] = 1.0
    esel = esel.astype(NP_BF16)

    in_maps = []
    for c in range(NCORES):
        q0 = c * CD
        wq = Wa[:, q0 : q0 + CD]
        wk = Wa[:, D + q0 : D + q0 + CD]
        wv = Wa[:, 2 * D + q0 : 2 * D + q0 + CD]
        bq = ba[q0 : q0 + CD]
        bk = ba[D + q0 : D + q0 + CD]
        bv = ba[2 * D + q0 : 2 * D + q0 + CD]
        in_maps.append(
            {
                "hsT": hsT,
                "w_qkv": np.ascontiguousarray(
                    np.concatenate([wq, wk, wv], axis=1)
                ).astype(NP_BF16),
                "b_qkv": np.ascontiguousarray(np.stack([bq, bk, bv], axis=1)).astype(
                    np.float32
                ),
                "w_p": np.ascontiguousarray(Wp[q0 : q0 + CD, :]).astype(NP_BF16),
                "msk": msk,
                "iden_b": np.eye(P).astype(NP_BF16),
                "esel": esel,
            }
        )
    return in_maps


def run(inputs, trace=False):
    """Build+run the sharded kernel. Returns (full_output, BassKernelResults)."""
    in_maps = prepare_inputs(**inputs)
    nc = _get_nc()
    res = run_bass_kernel_spmd(
        nc, in_maps, core_ids=list(range(NCORES)), trace=trace
    )
    acc = np.zeros((S, D), dtype=np.float32)
    for c in range(NCORES):
        acc += np.asarray(res.results[c]["out"], dtype=np.float32)
    acc += np.asarray(inputs["b_proj"], dtype=np.float32)
    return acc, res


def kernel(**inputs):
    out, _ = run(inputs, trace=False)
    return out


# revision 44
# speedup vs baseline: 1.2129x; 1.0439x over previous
"""Causal self-attention (S=2048, D=1024, H=16) on 8 Trainium2 NeuronCores.

Sharding: tensor-parallel over heads. Core c owns heads 2c, 2c+1:
  - computes qT/kT/vT for its 128 qkv-columns from the full hidden_states
    (contraction layouts; vT is PE-transposed back to natural [s, j]),
  - runs causal attention for its 2 heads (attT = K.Q^T blocks, exp via
    ScalarE, denominators via a ones-column in the PV matmul),
  - projects each head against its W_proj row-slice and fuses the softmax
    normalization into the projection epilogue (per-partition 1/den scales
    computed with a custom-DVE fast reciprocal and broadcast with K=1
    matmuls from partitions 0/64),
  - outputs a partial [S, D] product; the host sums the 8 partials and
    adds b_proj.

The bulk matmuls run in bf16 (1 cycle/row); the unnormalized attention
outputs and denominators stay in fp32. Phase 2 is software-pipelined
(logits of group g overlap the PV matmuls of group g-1); the projection
(phase 3) is spread through phase 2 as soon as each query chunk's
normalizer is ready, so the PE stays dense (HAM stays at full clock) and
the output DMA overlaps compute instead of trailing it.
"""

import math
from contextlib import ExitStack

import numpy as np

import concourse.bacc as bacc
import concourse.hw_specs as _hw_specs
import concourse.mybir as mybir
import concourse.tile as tile
from concourse.bass_utils import run_bass_kernel_spmd

# The kernel's only ScalarE activations are Exp and Ln. Left to itself, bass
# assigns Exp to the `exp_and_others` table set and Ln to `natural_log`, so
# every Exp->Ln alternation reloads the activation tables (~1.3us, 9x per
# kernel, serializing ScalarE mid-attention). Restrict the selectable sets to
# `natural_log_exp_and_others` (which contains both) so exactly one table
# load is emitted. Set ids keep their act_info.json positions.
_orig_gat = _hw_specs.get_activation_tables


def _pinned_gat(arch):
    return {
        name: (fns if name == "natural_log_exp_and_others" else set())
        for name, fns in _orig_gat(arch).items()
    }


bacc.get_activation_tables = _pinned_gat

S, D, H = 2048, 1024, 16
HS = D // H  # 64 head size
P = 128
NCORES = 8
HPC = H // NCORES  # 2 heads per core
CD = HPC * HS  # 128 per-core head dims
KO = D // P  # 8 contraction tiles for the projections
NQC = S // 512  # 4 query chunks
NSC = S // P  # 16 sequence chunks of 128
SCALE = 1.0 / math.sqrt(S)

F32 = mybir.dt.float32
F32R = mybir.dt.float32r
BF16 = mybir.dt.bfloat16

try:
    import ml_dtypes

    NP_BF16 = ml_dtypes.bfloat16
except ImportError:  # pragma: no cover
    NP_BF16 = None


def _build():
    nc = bacc.Bacc(
        "TRN2", target_bir_lowering=False, debug=False, num_devices=NCORES
    )

    hsT = nc.dram_tensor("hsT", [D, S], BF16, kind="ExternalInput")
    w_qkv = nc.dram_tensor("w_qkv", [D, 3 * P], BF16, kind="ExternalInput")
    b_qkv = nc.dram_tensor("b_qkv", [P, 3], F32, kind="ExternalInput")
    w_p = nc.dram_tensor("w_p", [CD, D], BF16, kind="ExternalInput")
    msk = nc.dram_tensor("msk", [P, 896], BF16, kind="ExternalInput")
    iden_b = nc.dram_tensor("iden_b", [P, P], BF16, kind="ExternalInput")
    esel = nc.dram_tensor("esel", [65, P], BF16, kind="ExternalInput")
    out = nc.dram_tensor("out", [S, D], F32, kind="ExternalOutput")

    with (
        tile.TileContext(nc) as tc,
        ExitStack() as ctx,
        nc.allow_low_precision(reason="bf16 matmul pipeline"),
    ):
        const = ctx.enter_context(tc.tile_pool(name="const", bufs=1))
        work = ctx.enter_context(tc.tile_pool(name="work", bufs=2))
        pp = ctx.enter_context(tc.tile_pool(name="pp", bufs=1, space="PSUM"))

        # PSUM: 2x 2-bank slots for attention logits (the two heads of a
        # group alternate slots, so each head's next logits launch while the
        # other head's exp drains), 2x 1-bank slots for everything else
        # (qkv passes, v transposes, projection halves, normalizer
        # broadcast), 2 banks for the per-chunk PV accumulators = all 8
        def att_ps(name):
            return pp.tile([P, 2, 512], F32, tag="attA", bufs=2, name=name)

        def b_ps(name):
            return pp.tile([P, 512], F32, tag="B", bufs=2, name=name)

        # ---- loads: identity first (pre-warm), then per-o weight+hsT chunks
        identb = const.tile([P, P], BF16, tag="identb", name="identb")
        nc.sync.dma_start(out=identb, in_=iden_b.ap())
        esel_sb = const.tile([65, P], BF16, tag="esel", name="esel_sb")
        nc.gpsimd.dma_start(out=esel_sb, in_=esel.ap())
        bqkv_sb = const.tile([P, 3], F32, tag="bqkv", name="bqkv_sb")
        nc.sync.dma_start(out=bqkv_sb, in_=b_qkv.ap())

        # input loads ride three DMA queues: sync + gpsimd + scalar (the
        # scalar engine is a HWDGE and sits idle until the first exp). The
        # phase-1 critical inputs (hsT columns 0:512 + the qkv weights) get
        # dedicated queues so nothing delays the first attention chunk.
        hsT_sb = const.tile([P, KO, S], BF16, tag="hsT", name="hsT_sb")
        wqkv_sb = const.tile([P, KO, 3 * P], BF16, tag="wqkv", name="wqkv_sb")
        for o in range(KO):
            nc.gpsimd.dma_start(
                out=wqkv_sb[:, o, :], in_=w_qkv.ap()[o * P : (o + 1) * P, :]
            )
            eng = nc.sync if o % 2 == 0 else nc.scalar
            eng.dma_start(
                out=hsT_sb[:, o, 0:512], in_=hsT.ap()[o * P : (o + 1) * P, 0:512]
            )
        msk_sb = const.tile([P, 896], BF16, tag="msk", name="msk_sb")
        nc.sync.dma_start(out=msk_sb, in_=msk.ap())
        wp_sb = const.tile([P, D], BF16, tag="wp", name="wp_sb")
        nc.gpsimd.dma_start(out=wp_sb, in_=w_p.ap())
        v_sb = []
        for h in range(HPC):
            vt = const.tile([P, NSC, HS + 1], BF16, tag=f"v{h}", name=f"v{h}_sb")
            nc.vector.memset(vt[:, :, HS], 1.0)  # ones column -> denominators
            v_sb.append(vt)
        # remaining hsT columns: wide (1.5KB-line) DMAs round-robin on 3 queues
        dma_rr = [nc.sync, nc.gpsimd, nc.scalar]
        for o in range(KO):
            dma_rr[(2 * o) % 3].dma_start(
                out=hsT_sb[:, o, 512:1280],
                in_=hsT.ap()[o * P : (o + 1) * P, 512:1280],
            )
            dma_rr[(2 * o + 1) % 3].dma_start(
                out=hsT_sb[:, o, 1280:2048],
                in_=hsT.ap()[o * P : (o + 1) * P, 1280:2048],
            )

        # pre-load the exp/ln activation table during the input DMA window
        # instead of at first use mid-attention (emitted after the scalar
        # queue's input DMA triggers so the table load doesn't delay them)
        scratch8 = const.tile([1, 8], F32, tag="scratch8", name="scratch8")
        nc.vector.memset(scratch8, 1.0)
        nc.scalar.activation(
            out=scratch8, in_=scratch8, func=mybir.ActivationFunctionType.Exp
        )

        qkT_sb = const.tile([P, 2, S], BF16, tag="qkT", name="qkT_sb")
        vT_sb = const.tile([P, S], BF16, tag="vT", name="vT_sb")
        u2_sb = [
            const.tile([P, 512], BF16, tag=f"u2_{qc}", name=f"u2_{qc}")
            for qc in range(NQC)
        ]
        u2n_sb = [
            const.tile([P, 512], BF16, tag=f"u2n_{qc}", name=f"u2n_{qc}")
            for qc in range(NQC)
        ]
        # softmax denominators for both heads: head h lives on partition 64h;
        # rows 1-63 are never written, so pre-fill with 1.0 to keep NaNs out
        # of the reciprocal -> selector-matmul path
        den_sb = []
        for qc in range(NQC):
            dt_ = const.tile([65, 512], F32, tag=f"den_{qc}", name=f"den_{qc}")
            nc.vector.memset(dt_, 1.0)
            den_sb.append(dt_)

        # ---- pre-warm the PE clock while the DMAs stream ---------------------
        # each burst consumes a freshly-arrived hsT chunk so the bursts are
        # spread across the load instead of back-to-back at t=0
        ps_w = b_ps("ps_w")
        for o in range(KO):
            for rep in range(2):
                nc.tensor.matmul(
                    ps_w,
                    lhsT=identb,
                    rhs=hsT_sb[:, o, 0:512],
                    start=True,
                    stop=True,
                )

        # ---- phase 1: qT, kT, vT ([j, s] layout) + v transposes -------------
        # split into 7 pieces per 512-chunk (3 qkv passes + 4 v transposes)
        # that get sprinkled between attention groups: the PE FIFO then
        # alternates phase-1 and attention work, so ScalarE's exp stream
        # never starves behind a phase-1 lump at chunk boundaries
        def emit_p1_m(n, m):
            ps_qkv = b_ps("ps_qkv")
            for o in range(KO):
                nc.tensor.matmul(
                    ps_qkv,
                    lhsT=wqkv_sb[:, o, m * P : (m + 1) * P],
                    rhs=hsT_sb[:, o, n * 512 : (n + 1) * 512],
                    start=(o == 0),
                    stop=(o == KO - 1),
                )
            dst = (
                qkT_sb[:, m, n * 512 : (n + 1) * 512]
                if m < 2
                else vT_sb[:, n * 512 : (n + 1) * 512]
            )
            nc.vector.tensor_scalar_add(
                out=dst, in0=ps_qkv, scalar1=bqkv_sb[:, m : m + 1]
            )

        def emit_p1_t(n, i):
            sc = 4 * n + i
            ps_t = pp.tile([P, P], BF16, tag="B", bufs=2, name="ps_t")
            nc.tensor.transpose(ps_t, vT_sb[:, sc * P : (sc + 1) * P], identb)
            for h in range(HPC):
                nc.vector.tensor_copy(
                    out=v_sb[h][:, sc, 0:HS], in_=ps_t[:, h * HS : (h + 1) * HS]
                )

        def p1_pieces(n):
            for m in range(3):
                yield lambda m=m: emit_p1_m(n, m)
            for i in range(4):
                yield lambda i=i: emit_p1_t(n, i)

        for piece in p1_pieces(0):
            piece()

        def emit_norm(qc):
            # 1/den = exp(-ln(den)) on ScalarE, both heads in one [65,512]
            # pass (the pinned table set holds Exp AND Ln: no table reloads);
            # one K=65 selector matmul broadcasts head h's reciprocal row to
            # its 64 partitions; one multiply normalizes both heads
            lnw = work.tile([65, 512], F32, tag="lnw", bufs=2, name="lnw")
            nc.scalar.activation(
                out=lnw, in_=den_sb[qc], func=mybir.ActivationFunctionType.Ln
            )
            rrec = work.tile([65, 512], BF16, tag="rrec", bufs=2, name="rrec")
            nc.scalar.activation(
                out=rrec,
                in_=lnw,
                func=mybir.ActivationFunctionType.Exp,
                scale=-1.0,
            )
            rb_ps = b_ps("ps_rb")
            nc.tensor.matmul(
                rb_ps, lhsT=esel_sb, rhs=rrec, start=True, stop=True
            )
            nc.vector.tensor_mul(out=u2n_sb[qc], in0=u2_sb[qc], in1=rb_ps)

        # ---- phase 3: projection over both heads (K=128), two 1-bank halves
        # per chunk so the B pool recycles quickly
        def emit_p3(sc, tail=False):
            qc = sc // 4
            f = sc % 4
            out_t = work.tile([P, 2, 512], F32, tag="out", bufs=4, name="out_t")
            for dc in range(2):
                slot = b_ps("ps_p3")
                nc.tensor.matmul(
                    slot,
                    lhsT=u2n_sb[qc][:, f * P : (f + 1) * P],
                    rhs=wp_sb[:, dc * 512 : (dc + 1) * 512],
                    start=True,
                    stop=True,
                )
                # in the tail the scalar engine is idle (no more exps): give
                # it half the PSUM evacuations
                if tail and dc == 0:
                    nc.scalar.copy(out=out_t[:, dc, :], in_=slot)
                else:
                    nc.vector.tensor_copy(out=out_t[:, dc, :], in_=slot)
            eng = (nc.sync, nc.gpsimd, nc.scalar)[sc % 3 if tail else sc % 2]
            eng.dma_start(
                out=out.ap()[sc * P : (sc + 1) * P, :],
                in_=out_t.rearrange("p a b -> p (a b)"),
            )

        # ---- phase 2: causal attention, software-pipelined ------------------
        for qc in range(NQC):
            ps_o = [
                pp.tile([P, 512], F32, tag="O", bufs=2, name=f"ps_o{h}")
                for h in range(HPC)
            ]
            nkb = 4 * (qc + 1)  # 128-wide key blocks in the causal span
            ngrp = nkb // 2
            # previous chunk's normalizer first: the PE has independent work
            # (p1 pieces, projections) to stay dense while Ln waits for the
            # denominator copies, and the exp stream restarts right after
            if qc >= 1:
                emit_norm(qc - 1)
            pieces = list(p1_pieces(qc + 1)) if qc + 1 < NQC else []
            npc = 0

            def emit_pv(pend, nkb=nkb, ps_o=ps_o):
                pes, kbs, f0 = pend
                for h in range(HPC):
                    for j, kb in enumerate(kbs):
                        nc.tensor.matmul(
                            ps_o[h][0 : HS + 1, f0:512],
                            lhsT=v_sb[h][:, kb, :],
                            rhs=pes[h][:, j, f0:512],
                            start=(kb == 0),
                            stop=(kb == nkb - 1),
                        )

            pending = None  # exp'd logits awaiting their PV matmuls
            for g in range(ngrp):
                kbs = [2 * g, 2 * g + 1]
                # last group covers only the causal upper half of the q range
                f0 = 256 if g == ngrp - 1 else 0
                # logits for both heads; explicit row-group tile positions
                # let the two K=64 matmuls run concurrently in disjoint
                # halves of the PE array
                ps_att = [att_ps(f"ps_att{h}") for h in range(HPC)]
                for j, kb in enumerate(kbs):
                    for h in range(HPC):
                        nc.tensor.matmul(
                            ps_att[h][:, j, f0:512],
                            lhsT=qkT_sb[h * HS : (h + 1) * HS, 1, kb * P : (kb + 1) * P],
                            rhs=qkT_sb[h * HS : (h + 1) * HS, 0, qc * 512 + f0 : (qc + 1) * 512],
                            start=True,
                            stop=True,
                            tile_position=(HS * h, 0),
                        )
                if pending is not None:
                    emit_pv(pending)
                pes = []
                for h in range(HPC):
                    p_exp = work.tile(
                        [P, 2, 512], BF16, tag=f"pe{h}", bufs=4, name="p_exp"
                    )
                    nc.scalar.activation(
                        out=p_exp[:, :, f0:512],
                        in_=ps_att[h][:, :, f0:512],
                        func=mybir.ActivationFunctionType.Exp,
                        scale=SCALE,
                    )
                    for j, kb in enumerate(kbs):
                        jj = kb - 4 * qc
                        if jj >= 0:  # diagonal block: causal 0/1 mask
                            off = 384 - 128 * jj
                            nc.vector.tensor_mul(
                                out=p_exp[:, j, f0:512],
                                in0=p_exp[:, j, f0:512],
                                in1=msk_sb[:, off + f0 : off + 512],
                            )
                    pes.append(p_exp)
                pending = (pes, kbs, f0)
                if qc >= 1 and g < 4:
                    emit_p3(4 * (qc - 1) + g)
                while npc < ((g + 1) * len(pieces)) // ngrp:
                    pieces[npc]()
                    npc += 1
            emit_pv(pending)

            # stash denominator rows first (they gate the next normalizer on
            # ScalarE), then the unnormalized head outputs; frees PSUM
            for h in range(HPC):
                nc.vector.tensor_copy(
                    out=den_sb[qc][64 * h : 64 * h + 1, :],
                    in_=ps_o[h][HS : HS + 1, :],
                )
            for h in range(HPC):
                nc.vector.tensor_copy(
                    out=u2_sb[qc][h * HS : (h + 1) * HS, :], in_=ps_o[h][0:HS, :]
                )
        # keep the PE busy through the final normalizer chain so the tail
        # projections run at full clock (HAM stays at 8/8)
        ps_warm = b_ps("ps_warm")
        for rep in range(6):
            nc.tensor.matmul(
                ps_warm,
                lhsT=identb,
                rhs=hsT_sb[:, rep, 0:512],
                start=True,
                stop=True,
            )
        emit_norm(3)
        for sc in range(12, NSC):
            emit_p3(sc, tail=True)

    nc.compile()
    return nc


_NC = None


def _get_nc():
    global _NC
    if _NC is None:
        _NC = _build()
    return _NC


def prepare_inputs(hidden_states, W_attn, b_attn, W_proj, b_proj):
    hs = np.asarray(hidden_states, dtype=np.float32)
    Wa = np.asarray(W_attn, dtype=np.float32)
    ba = np.asarray(b_attn, dtype=np.float32)
    Wp = np.asarray(W_proj, dtype=np.float32)

    hsT = np.ascontiguousarray(hs.T).astype(NP_BF16)
    pcol = np.arange(P)[:, None]
    ccol = np.arange(896)[None, :]
    msk = (pcol <= ccol - 384).astype(NP_BF16)
    esel = np.zeros((65, P), dtype=np.float32)
    esel[0, 0:HS] = 1.0
    esel[64, HS:P] = 1.0
    esel = esel.astype(NP_BF16)

    in_maps = []
    for c in range(NCORES):
        q0 = c * CD
        wq = Wa[:, q0 : q0 + CD]
        wk = Wa[:, D + q0 : D + q0 + CD]
        wv = Wa[:, 2 * D + q0 : 2 * D + q0 + CD]
        bq = ba[q0 : q0 + CD]
        bk = ba[D + q0 : D + q0 + CD]
        bv = ba[2 * D + q0 : 2 * D + q0 + CD]
        in_maps.append(
            {
                "hsT": hsT,
                "w_qkv": np.ascontiguousarray(
                    np.concatenate([wq, wk, wv], axis=1)
                ).astype(NP_BF16),
                "b_qkv": np.ascontiguousarray(np.stack([bq, bk, bv], axis=1)).astype(
                    np.float32
                ),
                "w_p": np.ascontiguousarray(Wp[q0 : q0 + CD, :]).astype(NP_BF16),
                "msk": msk,
                "iden_b": np.eye(P).astype(NP_BF16),
                "esel": esel,
            }
        )
    return in_maps


def run(inputs, trace=False):
    """Build+run the sharded kernel. Returns (full_output, BassKernelResults)."""
    in_maps = prepare_inputs(**inputs)
    nc = _get_nc()
    res = run_bass_kernel_spmd(
        nc, in_maps, core_ids=list(range(NCORES)), trace=trace
    )
    acc = np.zeros((S, D), dtype=np.float32)
    for c in range(NCORES):
        acc += np.asarray(res.results[c]["out"], dtype=np.float32)
    acc += np.asarray(inputs["b_proj"], dtype=np.float32)
    return acc, res


def kernel(**inputs):
    out, _ = run(inputs, trace=False)
    return out
